# revision 1
# baseline (speedup 1.0000x reference)
"""Causal self-attention (GQA + RMS-norm + partial RoPE) Trainium2 kernel.

Full inputs in, full output out. Sharding: 8 cores = batch(4) x head-half(2).
Each core computes its batch's QKV for 8 q-heads / 2 kv-heads entirely in
transposed layouts (head_dim on partitions), does causal attention with a
no-max softmax (scores bounded by sqrt(hd) after RMS norm), and a row-sharded
output projection; the host sums the two half partials per batch.

All large matmuls run in float32r (TF32-like, full PE rate at N=512).
Single long-lived PSUM pool with 8 rotating bank tags (t0..t7) — no
pool-scope barriers anywhere in the hot path.
"""
import numpy as np

import concourse.bacc as bacc
import concourse.mybir as mybir
from concourse.tile import TileContext
from concourse.bass_utils import run_bass_kernel_spmd

F32 = mybir.dt.float32
F32R = mybir.dt.float32r
AF = mybir.ActivationFunctionType

B, S, D = 4, 2048, 2048
H, KV, HD = 16, 4, 128
ROPE, HALF_ROPE = 64, 32
EPS = 1.1920929e-07
N_CORES = 8
NDC = D // 128          # 16 contraction chunks
NQC = S // 512          # 4 query chunks of 512
LH = 8                  # local q heads per core
LKV = 2                 # local kv heads per core

_cached_program = None
_last_in_maps = None


def _build_program():
    nc = bacc.Bacc("TRN2")
    # eps const AP for activation bias
    t = nc.alloc_sbuf_tensor("const-f32-eps", [128, 1], F32)
    nc.gpsimd.memset(t.ap(), EPS)
    nc.const_aps.aps[(F32, EPS)] = t.ap()
    nc.all_engine_barrier()

    xT = nc.declare_dram_parameter("xT", [D, S], F32R, isOutput=False)
    wqT = nc.declare_dram_parameter("wqT", [D, LH * HD], F32R, isOutput=False)
    wkT = nc.declare_dram_parameter("wkT", [D, LKV * HD], F32R, isOutput=False)
    wvT = nc.declare_dram_parameter("wvT", [D, LKV * HD], F32R, isOutput=False)
    wpT = nc.declare_dram_parameter("wpT", [LH * HD, D], F32R, isOutput=False)
    cosT = nc.declare_dram_parameter("cosT", [HALF_ROPE, S], F32, isOutput=False)
    sinT = nc.declare_dram_parameter("sinT", [HALF_ROPE, S], F32, isOutput=False)
    o128d = nc.declare_dram_parameter("ones128", [128, 1], F32R, isOutput=False)
    obd = nc.declare_dram_parameter("onesb", [1, 128], F32R, isOutput=False)
    gaind = nc.declare_dram_parameter("gains", [128, LH], F32, isOutput=False)
    out = nc.declare_dram_parameter("out", [S, D], F32, isOutput=True)

    with TileContext(nc) as tc:
        with (
            tc.tile_pool(name="cp", bufs=1) as cp,
            tc.tile_pool(name="xap", bufs=1) as xap,
            tc.tile_pool(name="wqp", bufs=2) as wqp,
            tc.tile_pool(name="wpp", bufs=2) as wpp,
            tc.tile_pool(name="stgp", bufs=2) as stgp,
            tc.tile_pool(name="scr", bufs=2) as scr,
            tc.tile_pool(name="exp3", bufs=3) as exp3,
            tc.tile_pool(name="qnp", bufs=1) as qnp,
            tc.tile_pool(name="pu", bufs=1, space="PSUM") as pu,
        ):
            wk_t = cp.tile([128, NDC, LKV * HD], F32R, tag="wk")
            nc.sync.dma_start(out=wk_t[:],
                              in_=wkT.rearrange("(c p) e -> p c e", p=128))
            wv_t = cp.tile([128, NDC, LKV * HD], F32R, tag="wv")
            nc.sync.dma_start(out=wv_t[:],
                              in_=wvT.rearrange("(c p) e -> p c e", p=128))
            cos_t = cp.tile([HALF_ROPE, S], F32, tag="cos")
            nc.sync.dma_start(out=cos_t[:], in_=cosT[:])
            sin_t = cp.tile([HALF_ROPE, S], F32, tag="sin")
            nc.sync.dma_start(out=sin_t[:], in_=sinT[:])
            o128 = cp.tile([128, 1], F32R, tag="o128")
            nc.sync.dma_start(out=o128[:], in_=o128d[:])
            ob = cp.tile([1, 128], F32R, tag="ob")
            nc.sync.dma_start(out=ob[:], in_=obd[:])
            gains = cp.tile([128, LH], F32, tag="gains")
            nc.sync.dma_start(out=gains[:], in_=gaind[:])
            kn_t = cp.tile([128, LKV, S], F32R, tag="kn")
            v_t = cp.tile([128, S // 128, LKV * HD], F32R, tag="v")

            def bank(i, shape=(128, 512), dt=F32, nm=None):
                return pu.tile(list(shape), dt, tag=f"t{i}",
                               name=nm or f"pt{i}")

            def load_x(pos0):
                """x chunk [128, NDC, 512] as 16 per-dc DMAs (compute on
                chunk dc can start as soon as its DMA lands)."""
                xa = xap.tile([128, NDC, 512], F32R, tag="xa", name="xa")
                for dc in range(NDC):
                    nc.sync.dma_start(
                        out=xa[:, dc],
                        in_=xT[dc * 128:(dc + 1) * 128, pos0:pos0 + 512])
                return xa

            def norm_rope(raw, out_ap, pos0):
                """RMS-norm + partial RoPE: transposed raw [128,512] (PSUM)
                -> out_ap ([128,512] f32r). pos0 selects cos/sin columns."""
                cs = slice(pos0, pos0 + 512)
                sq = scr.tile([128, 512], F32R, tag="sq", name="sq")
                nc.scalar.activation(sq[:], raw[:], AF.Square)
                ssq = bank(6, (1, 512), nm="ssq")
                nc.tensor.matmul(ssq[:], o128[:], sq[:], start=True, stop=True)
                # r = rsqrt(ssq/128 + eps) = sqrt(1/(ssq/128 + eps))
                u = scr.tile([1, 512], F32, tag="u", name="u")
                nc.scalar.activation(u[:], ssq[:], AF.Copy,
                                     scale=1.0 / HD, bias=EPS)
                uin = scr.tile([1, 512], F32, tag="uin", name="uin")
                nc.vector.reciprocal_approx_fast(out=uin[:], in_=u[:])
                r = scr.tile([1, 512], F32R, tag="rr", name="rr")
                nc.scalar.activation(r[:], uin[:], AF.Sqrt)
                Rp = bank(7, nm="Rp")
                nc.tensor.matmul(Rp[:], ob[:], r[:], start=True, stop=True)
                # cr/sr read R straight from PSUM (1 psum operand is legal);
                # only the rope pass-through rows need an SBUF copy of R
                Rsb = scr.tile([128, 512], F32, tag="Rsb", name="Rsb")
                nc.scalar.copy(Rsb[ROPE:128, :], Rp[ROPE:128, :])
                cr = scr.tile([HALF_ROPE, 512], F32, tag="cr", name="cr")
                sr = scr.tile([HALF_ROPE, 512], F32, tag="sr", name="sr")
                nc.vector.tensor_mul(cr[:], cos_t[:, cs], Rp[0:HALF_ROPE, :])
                nc.vector.tensor_mul(sr[:], sin_t[:, cs], Rp[0:HALF_ROPE, :])
                tmp = scr.tile([ROPE, 512], F32, tag="tmp", name="tmp")
                h1, h2 = slice(0, HALF_ROPE), slice(HALF_ROPE, ROPE)
                nc.vector.tensor_mul(out_ap[h1, :], raw[h1, :], cr[:])
                nc.vector.tensor_mul(tmp[h1, :], raw[h2, :], sr[:])
                nc.vector.tensor_add(out_ap[h1, :], out_ap[h1, :], tmp[h1, :])
                nc.vector.tensor_mul(out_ap[h2, :], raw[h2, :], cr[:])
                nc.vector.tensor_mul(tmp[h2, :], raw[h1, :], sr[:])
                nc.vector.tensor_sub(out_ap[h2, :], out_ap[h2, :], tmp[h2, :])
                nc.vector.tensor_mul(out_ap[ROPE:128, :], raw[ROPE:128, :],
                                     Rsb[ROPE:128, :])

            # ---------------- Phase A: kT (norm+rope) and v ----------------
            for sc in range(NQC):
                xa = load_x(sc * 512)
                for g in range(LKV):
                    kacc = bank(g, nm=f"kacc{g}")
                    for dc in range(NDC):
                        nc.tensor.matmul(
                            kacc[:], wk_t[:, dc, g * HD:(g + 1) * HD],
                            xa[:, dc], start=(dc == 0), stop=(dc == NDC - 1))
                    norm_rope(kacc, kn_t[:, g, sc * 512:(sc + 1) * 512],
                              sc * 512)
                for st4 in range(4):
                    st = sc * 4 + st4
                    vacc = bank(2 + st4 % 2, (128, LKV * HD), nm=f"vacc{st4}")
                    for dc in range(NDC):
                        nc.tensor.matmul(
                            vacc[:], xa[:, dc, st4 * 128:(st4 + 1) * 128],
                            wv_t[:, dc], start=(dc == 0), stop=(dc == NDC - 1))
                    nc.vector.tensor_copy(v_t[:, st], vacc[:])

            # ------------- Phase C: per query chunk q/attn/proj -------------
            for qc in range(NQC):
                pos0 = qc * 512
                n_kt = (qc + 1) * 4

                # -- q projection (two groups of 4 heads) + norm + rope --
                xa = load_x(pos0)
                qn = {}
                for grp in range(2):
                    qraw = {}
                    for dc in range(NDC):
                        wqt = wqp.tile([128, 512], F32R, tag="wq", name="wq")
                        nc.sync.dma_start(
                            out=wqt[:],
                            in_=wqT[dc * 128:(dc + 1) * 128,
                                    grp * 512:(grp + 1) * 512])
                        for hh in range(4):
                            if dc == 0:
                                qraw[hh] = bank(hh, nm=f"qraw{hh}")
                            nc.tensor.matmul(
                                qraw[hh][:], wqt[:, hh * HD:(hh + 1) * HD],
                                xa[:, dc], start=(dc == 0),
                                stop=(dc == NDC - 1))
                    for hh in range(4):
                        h = grp * 4 + hh
                        qn[h] = qnp.tile([128, 512], F32R, tag=f"qn{h}",
                                         name=f"qn{h}")
                        norm_rope(qraw[hh], qn[h][:], pos0)

                # -- attention --
                yt_sb = {}
                for h in range(LH):
                    g = h // 4
                    yt_ps = bank(h % 2, nm=f"yt{h}")
                    l_ps = bank(2 + h % 2, (1, 512), nm=f"l{h}")
                    for kt in range(n_kt):
                        sc_ps = bank(4 + kt % 4, nm=f"sc{h}_{kt}")
                        nc.tensor.matmul(
                            sc_ps[:], kn_t[:, g, kt * 128:(kt + 1) * 128],
                            qn[h][:], start=True, stop=True)
                        ex = exp3.tile([128, 512], F32R, tag="ex", name="ex")
                        nc.scalar.activation(ex[:], sc_ps[:], AF.Exp,
                                             scale=gains[:, h:h + 1])
                        m = kt - qc * 4
                        if m >= 0:
                            # zero ex where key pos > query pos:
                            # keep iff n - p - 128*m >= 0
                            nc.gpsimd.affine_select(
                                out=ex[:], in_=ex[:],
                                compare_op=mybir.AluOpType.is_ge,
                                fill=0.0, base=-128 * m,
                                pattern=[[1, 512]], channel_multiplier=-1)
                        nc.tensor.matmul(
                            yt_ps[:], v_t[:, kt, g * HD:(g + 1) * HD], ex[:],
                            start=(kt == 0), stop=(kt == n_kt - 1))
                        nc.tensor.matmul(
                            l_ps[:], o128[:], ex[:], start=(kt == 0),
                            stop=(kt == n_kt - 1))
                    lf = scr.tile([1, 512], F32, tag="lf", name="lf")
                    nc.vector.reciprocal_approx_fast(out=lf[:], in_=l_ps[:])
                    linv = scr.tile([1, 512], F32R, tag="linv", name="linv")
                    nc.scalar.copy(linv[:], lf[:])
                    Li_ps = bank(4 + (n_kt + h) % 4, nm=f"Li{h}")
                    nc.tensor.matmul(Li_ps[:], ob[:], linv[:],
                                     start=True, stop=True)
                    Lsb = scr.tile([128, 512], F32, tag="Lsb", name="Lsb")
                    nc.vector.tensor_copy(Lsb[:], Li_ps[:])
                    yt_sb[h] = qnp.tile([128, 512], F32R, tag=f"yts{h}",
                                        name=f"yts{h}")
                    nc.vector.tensor_mul(yt_sb[h][:], yt_ps[:], Lsb[:])

                # -- output projection: out[s_q, j] += yT.T @ wpT --
                # 4 j-columns of 512; 4 psum accumulators (t4..t7) per jcol
                for jcol in range(4):
                    prs = {}
                    for h in range(LH):
                        wpt = wpp.tile([128, 512], F32R, tag="wp", name="wp")
                        nc.sync.dma_start(
                            out=wpt[:],
                            in_=wpT[h * 128:(h + 1) * 128,
                                    jcol * 512:(jcol + 1) * 512])
                        for st4 in range(4):
                            if h == 0:
                                prs[st4] = bank(4 + st4, nm=f"pr{jcol}{st4}")
                            nc.tensor.matmul(
                                prs[st4][:],
                                yt_sb[h][:, st4 * 128:(st4 + 1) * 128],
                                wpt[:], start=(h == 0), stop=(h == LH - 1))
                    for st4 in range(4):
                        stg = stgp.tile([128, 512], F32, tag="stg", name="stg")
                        nc.vector.tensor_copy(stg[:], prs[st4][:])
                        nc.sync.dma_start(
                            out=out[pos0 + st4 * 128:pos0 + (st4 + 1) * 128,
                                    jcol * 512:(jcol + 1) * 512],
                            in_=stg[:])
    nc.compile()
    return nc


def _rope_tables():
    inv = 1.0 / (10000.0 ** (np.arange(0, ROPE, 2, dtype=np.float64) / ROPE))
    fr = np.outer(np.arange(S, dtype=np.float64), inv)  # [S, 32]
    return (np.cos(fr).T.astype(np.float32).copy(),
            np.sin(fr).T.astype(np.float32).copy())


def kernel(x, Wq, Wk, Wv, Wproj, q_gain):
    global _cached_program, _last_in_maps
    x = np.ascontiguousarray(np.asarray(x, dtype=np.float32))
    Wq = np.asarray(Wq, dtype=np.float32)
    Wk = np.asarray(Wk, dtype=np.float32)
    Wv = np.asarray(Wv, dtype=np.float32)
    Wproj = np.asarray(Wproj, dtype=np.float32)
    q_gain = np.asarray(q_gain, dtype=np.float32)

    cosT, sinT = _rope_tables()
    ones128 = np.ones((128, 1), dtype=np.float32)
    onesb = np.ones((1, 128), dtype=np.float32)
    scale = 1.0 / np.sqrt(HD)

    in_maps = []
    for core in range(N_CORES):
        b, half = core // 2, core % 2
        g0 = half * LKV
        gains = np.repeat((q_gain[half * LH:(half + 1) * LH] * scale)
                          [None, :], 128, axis=0).astype(np.float32)
        in_maps.append({
            "xT": np.ascontiguousarray(x[b].T),
            "wqT": np.ascontiguousarray(
                Wq[half * LH * HD:(half + 1) * LH * HD, :].T),
            "wkT": np.ascontiguousarray(
                Wk[g0 * HD:(g0 + LKV) * HD, :].T),
            "wvT": np.ascontiguousarray(
                Wv[g0 * HD:(g0 + LKV) * HD, :].T),
            "wpT": np.ascontiguousarray(
                Wproj[:, half * LH * HD:(half + 1) * LH * HD].T),
            "cosT": cosT, "sinT": sinT,
            "ones128": ones128, "onesb": onesb, "gains": gains,
        })

    _last_in_maps = in_maps
    if _cached_program is None:
        _cached_program = _build_program()
    res = run_bass_kernel_spmd(_cached_program, in_maps, list(range(N_CORES)))

    out = np.empty((B, S, D), dtype=np.float32)
    for b in range(B):
        out[b] = res.results[2 * b]["out"] + res.results[2 * b + 1]["out"]
    return out



# revision 9
# speedup vs baseline: 1.8167x; 1.8167x over previous
"""Causal self-attention (GQA + RMS-norm + partial RoPE) Trainium2 kernel.

Full inputs in, full output out. Sharding: 8 cores = batch(4) x head-half(2).
Each core handles one batch and 8 q-heads / 2 kv-heads in transposed layouts
(head_dim on partitions). v2 design:

- All matmul operands bf16 (fp32 PSUM accumulate): enables fast weight load,
  DVE 2x modes, halves SBUF/DMA. Weights DMAed once and kept resident.
- Single activation table set (natural_log_exp_and_others): Exp, Ln, Square,
  Copy only. rsqrt(x) computed as exp(-0.5*ln(x)).
- Attention processes key chunks in PAIRS: scores for 2x128 keys land in one
  [128,2,512] PSUM tile (2 banks), one Exp call covers both; causal mask via
  one gpsimd affine_select per diagonal pair.
- Phases: P(proj+norm+rope per 512-pos chunk) / A(attention per q-chunk) /
  O(output projection), emitted P0 P1 A0 P2 A1 O0 P3 A2 O1 A3 O2 O3.
- PSUM: pairA/pairB [128,2,512] (2 banks each) + acc0-2 [128,512] rotating
  accumulators + vec1 [1,512] = 8 banks.
"""
import numpy as np
import ml_dtypes

import concourse.bacc as bacc
import concourse.mybir as mybir
from concourse.tile import TileContext
from concourse.bass_utils import run_bass_kernel_spmd

F32 = mybir.dt.float32
F32R = mybir.dt.float32r
BF16 = mybir.dt.bfloat16
AF = mybir.ActivationFunctionType

B, S, D = 4, 2048, 2048
H, KV, HD = 16, 4, 128
ROPE, HALF_ROPE = 64, 32
EPS = 1.1920929e-07
N_CORES = 8
NDC = D // 128          # 16 contraction chunks
NQC = S // 512          # 4 query chunks of 512
LH = 8                  # local q heads per core
LKV = 2                 # local kv heads per core

_cached_program = None
_last_in_maps = None


def _build_program():
    nc = bacc.Bacc("TRN2")
    t = nc.alloc_sbuf_tensor("const-f32-eps", [128, 1], F32)
    nc.gpsimd.memset(t.ap(), EPS)
    nc.const_aps.aps[(F32, EPS)] = t.ap()
    nc.all_engine_barrier()

    xT = nc.declare_dram_parameter("xT", [D, S], BF16, isOutput=False)
    wqT = nc.declare_dram_parameter("wqT", [D, LH * HD], BF16, isOutput=False)
    wkT = nc.declare_dram_parameter("wkT", [D, LKV * HD], BF16, isOutput=False)
    wvT = nc.declare_dram_parameter("wvT", [D, LKV * HD], BF16, isOutput=False)
    wpT = nc.declare_dram_parameter("wpT", [LH * HD, D], BF16, isOutput=False)
    ccatd = nc.declare_dram_parameter("ccat", [ROPE, S], BF16, isOutput=False)
    scatd = nc.declare_dram_parameter("scat", [ROPE, S], BF16, isOutput=False)
    o128d = nc.declare_dram_parameter("o128", [128, 1], BF16, isOutput=False)
    obd = nc.declare_dram_parameter("ob", [1, 128], BF16, isOutput=False)
    obfd = nc.declare_dram_parameter("obf", [1, 128], F32R, isOutput=False)
    gaind = nc.declare_dram_parameter("gains", [128, LH], F32, isOutput=False)
    out = nc.declare_dram_parameter("out", [S, D], BF16, isOutput=True)

    with TileContext(nc) as tc:
        with (
            tc.tile_pool(name="cp", bufs=1) as cp,
            tc.tile_pool(name="xap", bufs=2) as xap,
            tc.tile_pool(name="qnp", bufs=2) as qnp,
            tc.tile_pool(name="ytp", bufs=2) as ytp,
            tc.tile_pool(name="scr", bufs=2) as scr,
            tc.tile_pool(name="exp3", bufs=3) as exp3,
            tc.tile_pool(name="pu", bufs=1, space="PSUM") as pu,
        ):
            # ---------------- resident constants / weights ----------------
            wq_t = cp.tile([128, NDC, LH * HD], BF16, tag="wq")
            for dc in range(NDC):
                nc.sync.dma_start(out=wq_t[:, dc],
                                  in_=wqT[dc * 128:(dc + 1) * 128, :])
            wk_t = cp.tile([128, NDC, LKV * HD], BF16, tag="wk")
            nc.sync.dma_start(out=wk_t[:],
                              in_=wkT.rearrange("(c p) e -> p c e", p=128))
            wv_t = cp.tile([128, NDC, LKV * HD], BF16, tag="wv")
            nc.sync.dma_start(out=wv_t[:],
                              in_=wvT.rearrange("(c p) e -> p c e", p=128))
            wp_t = cp.tile([128, LH, D], BF16, tag="wp")
            for hh in range(LH):
                nc.sync.dma_start(out=wp_t[:, hh],
                                  in_=wpT[hh * 128:(hh + 1) * 128, :])
            ccat = cp.tile([ROPE, S], BF16, tag="ccat")
            nc.sync.dma_start(out=ccat[:], in_=ccatd[:])
            scat = cp.tile([ROPE, S], BF16, tag="scat")
            nc.sync.dma_start(out=scat[:], in_=scatd[:])
            o128 = cp.tile([128, 1], BF16, tag="o128")
            nc.sync.dma_start(out=o128[:], in_=o128d[:])
            ob = cp.tile([1, 128], BF16, tag="ob")
            nc.sync.dma_start(out=ob[:], in_=obd[:])
            obf = cp.tile([1, 128], F32R, tag="obf")
            nc.sync.dma_start(out=obf[:], in_=obfd[:])
            gains = cp.tile([128, LH], F32, tag="gains")
            nc.sync.dma_start(out=gains[:], in_=gaind[:])
            kn_t = cp.tile([128, LKV, S], BF16, tag="kn")
            v_t = cp.tile([128, S // 128, LKV * HD], BF16, tag="v")

            _rot = [0]

            def acc_tile(shape=(128, 512), nm="acc"):
                i = _rot[0] % 3
                _rot[0] += 1
                return pu.tile(list(shape), F32, tag=f"acc{i}", name=nm)

            def norm_rope(raw, dst_full, dst_r1, dst_r2, dst_r64, cs):
                """RMS-norm + partial RoPE, raw [128,512] PSUM f32 ->
                bf16 dst (already-allocated APs: full/rows0:32/32:64/0:64).
                cs = column slice into the S-wide rope tables."""
                sq = scr.tile([128, 512], BF16, tag="sq", name="sq")
                nc.scalar.activation(sq[:], raw[:], AF.Square)
                ssq = pu.tile([1, 512], F32, tag="vec1", name="ssq")
                nc.tensor.matmul(ssq[:], o128[:], sq[:], start=True, stop=True)
                lnu = scr.tile([1, 512], F32, tag="lnu", name="lnu")
                nc.scalar.activation(lnu[:], ssq[:], AF.Ln,
                                     scale=1.0 / HD, bias=EPS)
                rr = scr.tile([1, 512], BF16, tag="rr", name="rr")
                nc.scalar.activation(rr[:], lnu[:], AF.Exp, scale=-0.5)
                Rp = acc_tile(nm="Rp")
                nc.tensor.matmul(Rp[:], ob[:], rr[:], start=True, stop=True)
                Rb = scr.tile([128, 512], F32, tag="Rb", name="Rb")
                nc.vector.tensor_copy(Rb[:], Rp[:])
                # dst = raw * R (all 128 rows), then rope rows 0:64 in place
                nc.vector.tensor_mul(dst_full, raw[:], Rb[:])
                # scat rows 0:32 = -sin, rows 32:64 = +sin so each TT below
                # has equal SBUF base partitions for its two inputs
                tmp = scr.tile([ROPE, 512], BF16, tag="tmp", name="tmp")
                nc.vector.tensor_mul(tmp[0:HALF_ROPE, :], dst_r2,
                                     scat[HALF_ROPE:ROPE, cs])
                nc.vector.tensor_mul(tmp[HALF_ROPE:ROPE, :], dst_r1,
                                     scat[0:HALF_ROPE, cs])
                nc.vector.tensor_mul(dst_r64, dst_r64, ccat[:, cs])
                nc.vector.tensor_add(dst_r64, dst_r64, tmp[:])

            qn = {}

            def phaseP(sc):
                pos0 = sc * 512
                cs = slice(pos0, pos0 + 512)
                xa = xap.tile([128, NDC, 512], BF16, tag="xa", name="xa")
                for dc in range(NDC):
                    nc.sync.dma_start(
                        out=xa[:, dc],
                        in_=xT[dc * 128:(dc + 1) * 128, pos0:pos0 + 512])
                # K projections + norm/rope
                for g in range(LKV):
                    kraw = acc_tile(nm=f"kraw{g}")
                    for dc in range(NDC):
                        nc.tensor.matmul(
                            kraw[:], wk_t[:, dc, g * HD:(g + 1) * HD],
                            xa[:, dc], start=(dc == 0), stop=(dc == NDC - 1))
                    norm_rope(kraw, kn_t[:, g, cs],
                              kn_t[0:HALF_ROPE, g, cs],
                              kn_t[HALF_ROPE:ROPE, g, cs],
                              kn_t[0:ROPE, g, cs], cs)
                # Q projections + norm/rope, with V blocks interleaved
                for h in range(LH):
                    qt = qnp.tile([128, 512], BF16, tag=f"qn{h}",
                                  name=f"qn{h}")
                    qn[(sc, h)] = qt
                    qraw = acc_tile(nm=f"qraw{h}")
                    for dc in range(NDC):
                        nc.tensor.matmul(
                            qraw[:], wq_t[:, dc, h * HD:(h + 1) * HD],
                            xa[:, dc], start=(dc == 0), stop=(dc == NDC - 1))
                    norm_rope(qraw, qt[:], qt[0:HALF_ROPE, :],
                              qt[HALF_ROPE:ROPE, :], qt[0:ROPE, :], cs)
                    if h % 2 == 1:
                        st4 = h // 2
                        vacc = acc_tile((128, LKV * HD), nm=f"vacc{st4}")
                        for dc in range(NDC):
                            nc.tensor.matmul(
                                vacc[:], xa[:, dc, st4 * 128:(st4 + 1) * 128],
                                wv_t[:, dc], start=(dc == 0),
                                stop=(dc == NDC - 1))
                        nc.scalar.copy(v_t[:, sc * 4 + st4], vacc[:])

            yt_sb = {}

            def phaseA(qc):
                npair = 2 * (qc + 1)
                for h in range(LH):
                    g = h // 4
                    yt = acc_tile(nm=f"yt{h}")
                    lps = pu.tile([1, 512], F32, tag="vec1", name=f"l{h}")
                    # diagonal pairs first (their exp+mask chain is longest)
                    order = [2 * qc, 2 * qc + 1] + list(range(2 * qc))
                    pend = None  # (ex, j, first) awaiting yt/l emission

                    def flush(last):
                        ex, j, first = pend
                        for o in range(2):
                            nc.tensor.matmul(
                                yt[:], v_t[:, 2 * j + o, g * HD:(g + 1) * HD],
                                ex[:, o], start=(first and o == 0),
                                stop=(last and o == 1))
                            nc.tensor.matmul(
                                lps[:], o128[:], ex[:, o],
                                start=(first and o == 0),
                                stop=(last and o == 1))

                    for idx, j in enumerate(order):
                        pair = pu.tile([128, 2, 512], F32,
                                       tag=("pA" if idx % 2 == 0 else "pB"),
                                       name=f"p{h}_{j}")
                        for o in range(2):
                            nc.tensor.matmul(
                                pair[:, o],
                                kn_t[:, g, (2 * j + o) * 128:
                                     (2 * j + o + 1) * 128],
                                qn[(qc, h)][:], start=True, stop=True)
                        ex = exp3.tile([128, 2, 512], BF16, tag="ex",
                                       name="ex")
                        nc.scalar.activation(ex[:], pair[:], AF.Exp,
                                             scale=gains[:, h:h + 1])
                        if j >= 2 * qc:
                            # keep iff n - 128*o - p - 128*m_off >= 0
                            nc.gpsimd.affine_select(
                                out=ex[:], in_=ex[:],
                                compare_op=mybir.AluOpType.is_ge,
                                fill=0.0, base=-128 * (2 * j - 4 * qc),
                                pattern=[[-128, 2], [1, 512]],
                                channel_multiplier=-1)
                        if pend is not None:
                            flush(False)
                        pend = (ex, j, idx == 0)
                    flush(True)
                    # epilogue: yt_sb = yt / l
                    lf = scr.tile([1, 512], F32, tag="lf", name="lf")
                    nc.vector.reciprocal_approx_fast(out=lf[:], in_=lps[:])
                    linv = scr.tile([1, 512], F32R, tag="linv", name="linv")
                    nc.scalar.copy(linv[:], lf[:])
                    Li = acc_tile(nm=f"Li{h}")
                    nc.tensor.matmul(Li[:], obf[:], linv[:],
                                     start=True, stop=True)
                    Lsb = scr.tile([128, 512], F32, tag="Lsb", name="Lsb")
                    nc.vector.tensor_copy(Lsb[:], Li[:])
                    ys = ytp.tile([128, 512], BF16, tag=f"yts{h}",
                                  name=f"yts{h}")
                    yt_sb[(qc, h)] = ys
                    nc.vector.tensor_mul(ys[:], yt[:], Lsb[:])

            def phaseO(qc):
                pos0 = qc * 512
                for jcol in range(4):
                    for st4 in range(4):
                        prs = acc_tile(nm=f"pr{jcol}{st4}")
                        for h in range(LH):
                            nc.tensor.matmul(
                                prs[:],
                                yt_sb[(qc, h)][:, st4 * 128:(st4 + 1) * 128],
                                wp_t[:, h, jcol * 512:(jcol + 1) * 512],
                                start=(h == 0), stop=(h == LH - 1))
                        stg = scr.tile([128, 512], BF16, tag="stg",
                                       name="stg")
                        nc.scalar.copy(stg[:], prs[:])
                        nc.sync.dma_start(
                            out=out[pos0 + st4 * 128:pos0 + (st4 + 1) * 128,
                                    jcol * 512:(jcol + 1) * 512],
                            in_=stg[:])

            phaseP(0)
            phaseP(1)
            phaseA(0)
            phaseP(2)
            phaseA(1)
            phaseO(0)
            phaseP(3)
            phaseA(2)
            phaseO(1)
            phaseA(3)
            phaseO(2)
            phaseO(3)
    nc.compile()
    return nc


def _rope_tables():
    inv = 1.0 / (10000.0 ** (np.arange(0, ROPE, 2, dtype=np.float64) / ROPE))
    fr = np.outer(np.arange(S, dtype=np.float64), inv)  # [S, 32]
    cos = np.cos(fr).T  # [32, S]
    sin = np.sin(fr).T
    ccat = np.concatenate([cos, cos], axis=0)
    scat = np.concatenate([-sin, sin], axis=0)
    return (ccat.astype(ml_dtypes.bfloat16), scat.astype(ml_dtypes.bfloat16))


def kernel(x, Wq, Wk, Wv, Wproj, q_gain):
    global _cached_program, _last_in_maps
    x = np.asarray(x, dtype=np.float32)
    Wq = np.asarray(Wq, dtype=np.float32)
    Wk = np.asarray(Wk, dtype=np.float32)
    Wv = np.asarray(Wv, dtype=np.float32)
    Wproj = np.asarray(Wproj, dtype=np.float32)
    q_gain = np.asarray(q_gain, dtype=np.float32)

    ccat, scat = _rope_tables()
    o128 = np.ones((128, 1), dtype=ml_dtypes.bfloat16)
    ob = np.ones((1, 128), dtype=ml_dtypes.bfloat16)
    obf = np.ones((1, 128), dtype=np.float32)
    scale = 1.0 / np.sqrt(HD)

    bf = ml_dtypes.bfloat16
    in_maps = []
    for core in range(N_CORES):
        b, half = core // 2, core % 2
        g0 = half * LKV
        gains = np.repeat((q_gain[half * LH:(half + 1) * LH] * scale)
                          [None, :], 128, axis=0).astype(np.float32)
        in_maps.append({
            "xT": np.ascontiguousarray(x[b].T).astype(bf),
            "wqT": np.ascontiguousarray(
                Wq[half * LH * HD:(half + 1) * LH * HD, :].T).astype(bf),
            "wkT": np.ascontiguousarray(
                Wk[g0 * HD:(g0 + LKV) * HD, :].T).astype(bf),
            "wvT": np.ascontiguousarray(
                Wv[g0 * HD:(g0 + LKV) * HD, :].T).astype(bf),
            "wpT": np.ascontiguousarray(
                Wproj[:, half * LH * HD:(half + 1) * LH * HD].T).astype(bf),
            "ccat": ccat, "scat": scat,
            "o128": o128, "ob": ob, "obf": obf, "gains": gains,
        })

    _last_in_maps = in_maps
    if _cached_program is None:
        _cached_program = _build_program()
    res = run_bass_kernel_spmd(_cached_program, in_maps, list(range(N_CORES)))

    outp = np.empty((B, S, D), dtype=np.float32)
    for b in range(B):
        outp[b] = (res.results[2 * b]["out"].astype(np.float32)
                   + res.results[2 * b + 1]["out"].astype(np.float32))
    return outp


# revision 11
# speedup vs baseline: 2.2758x; 1.2527x over previous
"""Causal self-attention (GQA + RMS-norm + partial RoPE) Trainium2 kernel.

Full inputs in, full output out. Sharding: 8 cores = batch(4) x head-half(2).
Each core handles one batch and 8 q-heads / 2 kv-heads in transposed layouts
(head_dim on partitions). v3 design:

- All matmul operands bf16 (fp32 PSUM accumulate): fast weight load, DVE 2x
  modes, halved SBUF/DMA. Weights DMAed once and kept resident.
- Single activation table set (natural_log_exp_and_others): Exp, Ln, Square,
  Copy only; rsqrt(x) = exp(-0.5*ln(x)). Table chooser pinned to that set.
- R / 1/l broadcasts via gpsimd.partition_broadcast (no tensor-engine
  broadcast matmuls, no PSUM banks for them).
- Attention: key chunks in PAIRS ([128,2,512] PSUM, one Exp per pair);
  TWO heads interleaved so one head's exp/mask latency hides behind the
  other head's matmuls. Causal mask: one gpsimd affine_select per
  diagonal pair.
- PSUM banks: pA/pB [128,2,512] (2 each) + acc0/acc1 [128,512] +
  vec1a/vec1b [1,512] = 8.
"""
import numpy as np
import ml_dtypes

import concourse.bacc as bacc
import concourse.mybir as mybir
from concourse.tile import TileContext
from concourse.bass_utils import run_bass_kernel_spmd

# The ACT table-load inserter picks the FIRST act-function set covering each
# activation: Exp/Square/Copy -> exp_and_others(0) but Ln -> natural_log(5),
# so interleaved norm+softmax work thrashes table loads (~1.3us each).  All
# four functions we use coexist in natural_log_exp_and_others; steer the
# chooser there by hiding them from the coverage sets of every OTHER table.
# Set ids are unchanged, so the emitted program stays valid.
_AF = mybir.ActivationFunctionType
_PINNED_SET = "natural_log_exp_and_others"
_PINNED_FUNCS = {_AF.Exp, _AF.Ln, _AF.Square, _AF.Copy}
_orig_get_act_tables = bacc.get_activation_tables


def _pinned_get_act_tables(arch):
    tabs = _orig_get_act_tables(arch)
    return {
        name: (funcs if name == _PINNED_SET else funcs - _PINNED_FUNCS)
        for name, funcs in tabs.items()
    }


bacc.get_activation_tables = _pinned_get_act_tables

F32 = mybir.dt.float32
F32R = mybir.dt.float32r
BF16 = mybir.dt.bfloat16
AF = mybir.ActivationFunctionType

B, S, D = 4, 2048, 2048
H, KV, HD = 16, 4, 128
ROPE, HALF_ROPE = 64, 32
EPS = 1.1920929e-07
N_CORES = 8
NDC = D // 128          # 16 contraction chunks
NQC = S // 512          # 4 query chunks of 512
LH = 8                  # local q heads per core
LKV = 2                 # local kv heads per core

_cached_program = None
_last_in_maps = None


def _build_program():
    nc = bacc.Bacc("TRN2")
    t = nc.alloc_sbuf_tensor("const-f32-eps", [128, 1], F32)
    nc.gpsimd.memset(t.ap(), EPS)
    nc.const_aps.aps[(F32, EPS)] = t.ap()
    nc.all_engine_barrier()

    xT = nc.declare_dram_parameter("xT", [D, S], BF16, isOutput=False)
    wqT = nc.declare_dram_parameter("wqT", [D, LH * HD], BF16, isOutput=False)
    wkT = nc.declare_dram_parameter("wkT", [D, LKV * HD], BF16, isOutput=False)
    wvT = nc.declare_dram_parameter("wvT", [D, LKV * HD], BF16, isOutput=False)
    wpT = nc.declare_dram_parameter("wpT", [LH * HD, D], BF16, isOutput=False)
    ccatd = nc.declare_dram_parameter("ccat", [ROPE, S], BF16, isOutput=False)
    scatd = nc.declare_dram_parameter("scat", [ROPE, S], BF16, isOutput=False)
    o128d = nc.declare_dram_parameter("o128", [128, 1], BF16, isOutput=False)
    gaind = nc.declare_dram_parameter("gains", [128, LH], F32, isOutput=False)
    out = nc.declare_dram_parameter("out", [S, D], BF16, isOutput=True)

    with TileContext(nc) as tc:
        with (
            tc.tile_pool(name="cp", bufs=1) as cp,
            tc.tile_pool(name="xap", bufs=2) as xap,
            tc.tile_pool(name="qnp", bufs=2) as qnp,
            tc.tile_pool(name="ytp", bufs=2) as ytp,
            tc.tile_pool(name="scr", bufs=2) as scr,
            tc.tile_pool(name="exp4", bufs=4) as exp4,
            tc.tile_pool(name="pu", bufs=1, space="PSUM") as pu,
        ):
            # ---- constants / weights, DMA-ordered by first use ----
            o128 = cp.tile([128, 1], BF16, tag="o128")
            nc.sync.dma_start(out=o128[:], in_=o128d[:])
            gains = cp.tile([128, LH], F32, tag="gains")
            nc.sync.dma_start(out=gains[:], in_=gaind[:])
            ccat = cp.tile([ROPE, S], BF16, tag="ccat")
            nc.sync.dma_start(out=ccat[:], in_=ccatd[:])
            scat = cp.tile([ROPE, S], BF16, tag="scat")
            nc.sync.dma_start(out=scat[:], in_=scatd[:])
            wk_t = cp.tile([128, NDC, LKV * HD], BF16, tag="wk")
            nc.sync.dma_start(out=wk_t[:],
                              in_=wkT.rearrange("(c p) e -> p c e", p=128))
            xa0 = xap.tile([128, NDC, 512], BF16, tag="xa", name="xa")
            for dc in range(NDC):
                nc.sync.dma_start(out=xa0[:, dc],
                                  in_=xT[dc * 128:(dc + 1) * 128, 0:512])
            wq_t = cp.tile([128, NDC, LH * HD], BF16, tag="wq")
            for dc in range(NDC):
                nc.sync.dma_start(out=wq_t[:, dc],
                                  in_=wqT[dc * 128:(dc + 1) * 128, :])
            wv_t = cp.tile([128, NDC, LKV * HD], BF16, tag="wv")
            nc.sync.dma_start(out=wv_t[:],
                              in_=wvT.rearrange("(c p) e -> p c e", p=128))
            wp_t = cp.tile([128, LH, D], BF16, tag="wp")
            for hh in range(LH):
                nc.sync.dma_start(out=wp_t[:, hh],
                                  in_=wpT[hh * 128:(hh + 1) * 128, :])
            kn_t = cp.tile([128, LKV, S], BF16, tag="kn")
            v_t = cp.tile([128, S // 128, LKV * HD], BF16, tag="v")

            # PSUM accumulator rotation: acc0, acc1, plus the (wider) pA/pB
            # slots which the P/O phases may borrow as [128,512] tiles.
            _rot = [0]
            _ROT_TAGS = ["acc0", "acc1", "pA", "pB"]

            def acc_tile(shape=(128, 512), nm="acc"):
                tag = _ROT_TAGS[_rot[0] % 4]
                _rot[0] += 1
                return pu.tile(list(shape), F32, tag=tag, name=nm)

            _v1 = [0]

            def vec1_tile(nm):
                tag = "vec1a" if _v1[0] % 2 == 0 else "vec1b"
                _v1[0] += 1
                return pu.tile([1, 512], F32, tag=tag, name=nm)

            def norm_rope(raw, dst_full, dst_r1, dst_r2, dst_r64, cs):
                """RMS-norm + partial RoPE, raw [128,512] PSUM f32 ->
                bf16 dst (already-allocated APs: full/rows0:32/32:64/0:64).
                cs = column slice into the S-wide rope tables."""
                sq = scr.tile([128, 512], BF16, tag="sq", name="sq")
                nc.scalar.activation(sq[:], raw[:], AF.Square)
                ssq = vec1_tile("ssq")
                nc.tensor.matmul(ssq[:], o128[:], sq[:], start=True, stop=True)
                lnu = scr.tile([1, 512], F32, tag="lnu", name="lnu")
                nc.scalar.activation(lnu[:], ssq[:], AF.Ln,
                                     scale=1.0 / HD, bias=EPS)
                rr = scr.tile([1, 512], F32, tag="rr", name="rr")
                nc.scalar.activation(rr[:], lnu[:], AF.Exp, scale=-0.5)
                Rb = scr.tile([128, 512], F32, tag="Rb", name="Rb")
                nc.gpsimd.partition_broadcast(Rb[:], rr[:])
                nc.vector.tensor_mul(dst_full, raw[:], Rb[:])
                # scat rows 0:32 = -sin, rows 32:64 = +sin so each TT below
                # has equal SBUF base partitions for its two inputs
                tmp = scr.tile([ROPE, 512], BF16, tag="tmp", name="tmp")
                nc.vector.tensor_mul(tmp[0:HALF_ROPE, :], dst_r2,
                                     scat[HALF_ROPE:ROPE, cs])
                nc.vector.tensor_mul(tmp[HALF_ROPE:ROPE, :], dst_r1,
                                     scat[0:HALF_ROPE, cs])
                nc.vector.tensor_mul(dst_r64, dst_r64, ccat[:, cs])
                nc.vector.tensor_add(dst_r64, dst_r64, tmp[:])

            qn = {}

            def phaseP(sc):
                pos0 = sc * 512
                cs = slice(pos0, pos0 + 512)
                if sc == 0:
                    xa = xa0
                else:
                    xa = xap.tile([128, NDC, 512], BF16, tag="xa", name="xa")
                    for dc in range(NDC):
                        nc.sync.dma_start(
                            out=xa[:, dc],
                            in_=xT[dc * 128:(dc + 1) * 128, pos0:pos0 + 512])
                for g in range(LKV):
                    kraw = acc_tile(nm=f"kraw{g}")
                    for dc in range(NDC):
                        nc.tensor.matmul(
                            kraw[:], wk_t[:, dc, g * HD:(g + 1) * HD],
                            xa[:, dc], start=(dc == 0), stop=(dc == NDC - 1))
                    norm_rope(kraw, kn_t[:, g, cs],
                              kn_t[0:HALF_ROPE, g, cs],
                              kn_t[HALF_ROPE:ROPE, g, cs],
                              kn_t[0:ROPE, g, cs], cs)
                for h in range(LH):
                    qt = qnp.tile([128, 512], BF16, tag=f"qn{h}",
                                  name=f"qn{h}")
                    qn[(sc, h)] = qt
                    qraw = acc_tile(nm=f"qraw{h}")
                    for dc in range(NDC):
                        nc.tensor.matmul(
                            qraw[:], wq_t[:, dc, h * HD:(h + 1) * HD],
                            xa[:, dc], start=(dc == 0), stop=(dc == NDC - 1))
                    norm_rope(qraw, qt[:], qt[0:HALF_ROPE, :],
                              qt[HALF_ROPE:ROPE, :], qt[0:ROPE, :], cs)
                    if h % 2 == 1:
                        st4 = h // 2
                        vacc = acc_tile((128, LKV * HD), nm=f"vacc{st4}")
                        for dc in range(NDC):
                            nc.tensor.matmul(
                                vacc[:], xa[:, dc, st4 * 128:(st4 + 1) * 128],
                                wv_t[:, dc], start=(dc == 0),
                                stop=(dc == NDC - 1))
                        nc.scalar.copy(v_t[:, sc * 4 + st4], vacc[:])

            yt_sb = {}

            def phaseA(qc):
                npair = 2 * (qc + 1)
                # two heads interleaved; same kv group within each duo
                for duo in range(LH // 2):
                    hs = (2 * duo, 2 * duo + 1)
                    g = hs[0] // 4
                    yt = {}
                    lps = {}
                    pend = {}
                    for i, h in enumerate(hs):
                        yt[h] = pu.tile([128, 512], F32, tag=f"acc{i}",
                                        name=f"yt{h}")
                        lps[h] = vec1_tile(f"l{h}")
                        pend[h] = None

                    def flush(h, last):
                        ex, j, first = pend[h]
                        for o in range(2):
                            nc.tensor.matmul(
                                yt[h][:],
                                v_t[:, 2 * j + o, g * HD:(g + 1) * HD],
                                ex[:, o], start=(first and o == 0),
                                stop=(last and o == 1))
                            nc.tensor.matmul(
                                lps[h][:], o128[:], ex[:, o],
                                start=(first and o == 0),
                                stop=(last and o == 1))

                    # diagonal pairs first: longest exp+mask chains
                    order = [2 * qc, 2 * qc + 1] + list(range(2 * qc))
                    for idx, j in enumerate(order):
                        for i, h in enumerate(hs):
                            pair = pu.tile([128, 2, 512], F32,
                                           tag=("pA" if i == 0 else "pB"),
                                           name=f"p{h}_{j}")
                            for o in range(2):
                                nc.tensor.matmul(
                                    pair[:, o],
                                    kn_t[:, g, (2 * j + o) * 128:
                                         (2 * j + o + 1) * 128],
                                    qn[(qc, h)][:], start=True, stop=True)
                            ex = exp4.tile([128, 2, 512], BF16, tag="ex",
                                           name="ex")
                            nc.scalar.activation(ex[:], pair[:], AF.Exp,
                                                 scale=gains[:, h:h + 1])
                            if j >= 2 * qc:
                                # keep iff n - 128*o - p - 128*m_off >= 0
                                nc.gpsimd.affine_select(
                                    out=ex[:], in_=ex[:],
                                    compare_op=mybir.AluOpType.is_ge,
                                    fill=0.0, base=-128 * (2 * j - 4 * qc),
                                    pattern=[[-128, 2], [1, 512]],
                                    channel_multiplier=-1)
                            if pend[h] is not None:
                                flush(h, False)
                            pend[h] = (ex, j, idx == 0)
                    for h in hs:
                        flush(h, True)
                        lf = scr.tile([1, 512], F32, tag="lf", name="lf")
                        nc.vector.reciprocal_approx_fast(out=lf[:],
                                                         in_=lps[h][:])
                        Lsb = scr.tile([128, 512], F32, tag="Lsb", name="Lsb")
                        nc.gpsimd.partition_broadcast(Lsb[:], lf[:])
                        ys = ytp.tile([128, 512], BF16, tag=f"yts{h}",
                                      name=f"yts{h}")
                        yt_sb[(qc, h)] = ys
                        nc.vector.tensor_mul(ys[:], yt[h][:], Lsb[:])

            def phaseO(qc):
                pos0 = qc * 512
                for jcol in range(4):
                    for st4 in range(4):
                        prs = acc_tile(nm=f"pr{jcol}{st4}")
                        for h in range(LH):
                            nc.tensor.matmul(
                                prs[:],
                                yt_sb[(qc, h)][:, st4 * 128:(st4 + 1) * 128],
                                wp_t[:, h, jcol * 512:(jcol + 1) * 512],
                                start=(h == 0), stop=(h == LH - 1))
                        stg = scr.tile([128, 512], BF16, tag="stg",
                                       name="stg")
                        nc.scalar.copy(stg[:], prs[:])
                        nc.sync.dma_start(
                            out=out[pos0 + st4 * 128:pos0 + (st4 + 1) * 128,
                                    jcol * 512:(jcol + 1) * 512],
                            in_=stg[:])

            phaseP(0)
            phaseP(1)
            phaseA(0)
            phaseP(2)
            phaseA(1)
            phaseO(0)
            phaseP(3)
            phaseA(2)
            phaseO(1)
            phaseA(3)
            phaseO(2)
            phaseO(3)
    nc.compile()
    return nc


def _rope_tables():
    inv = 1.0 / (10000.0 ** (np.arange(0, ROPE, 2, dtype=np.float64) / ROPE))
    fr = np.outer(np.arange(S, dtype=np.float64), inv)  # [S, 32]
    cos = np.cos(fr).T  # [32, S]
    sin = np.sin(fr).T
    ccat = np.concatenate([cos, cos], axis=0)
    scat = np.concatenate([-sin, sin], axis=0)
    return (ccat.astype(ml_dtypes.bfloat16), scat.astype(ml_dtypes.bfloat16))


def kernel(x, Wq, Wk, Wv, Wproj, q_gain):
    global _cached_program, _last_in_maps
    x = np.asarray(x, dtype=np.float32)
    Wq = np.asarray(Wq, dtype=np.float32)
    Wk = np.asarray(Wk, dtype=np.float32)
    Wv = np.asarray(Wv, dtype=np.float32)
    Wproj = np.asarray(Wproj, dtype=np.float32)
    q_gain = np.asarray(q_gain, dtype=np.float32)

    ccat, scat = _rope_tables()
    o128 = np.ones((128, 1), dtype=ml_dtypes.bfloat16)
    scale = 1.0 / np.sqrt(HD)

    bf = ml_dtypes.bfloat16
    in_maps = []
    for core in range(N_CORES):
        b, half = core // 2, core % 2
        g0 = half * LKV
        gains = np.repeat((q_gain[half * LH:(half + 1) * LH] * scale)
                          [None, :], 128, axis=0).astype(np.float32)
        in_maps.append({
            "xT": np.ascontiguousarray(x[b].T).astype(bf),
            "wqT": np.ascontiguousarray(
                Wq[half * LH * HD:(half + 1) * LH * HD, :].T).astype(bf),
            "wkT": np.ascontiguousarray(
                Wk[g0 * HD:(g0 + LKV) * HD, :].T).astype(bf),
            "wvT": np.ascontiguousarray(
                Wv[g0 * HD:(g0 + LKV) * HD, :].T).astype(bf),
            "wpT": np.ascontiguousarray(
                Wproj[:, half * LH * HD:(half + 1) * LH * HD].T).astype(bf),
            "ccat": ccat, "scat": scat,
            "o128": o128, "gains": gains,
        })

    _last_in_maps = in_maps
    if _cached_program is None:
        _cached_program = _build_program()
    res = run_bass_kernel_spmd(_cached_program, in_maps, list(range(N_CORES)))

    outp = np.empty((B, S, D), dtype=np.float32)
    for b in range(B):
        outp[b] = (res.results[2 * b]["out"].astype(np.float32)
                   + res.results[2 * b + 1]["out"].astype(np.float32))
    return outp
